# revision 1
# baseline (speedup 1.0000x reference)
"""Trainium2 Bass kernel for nn_LRSVConv (low-rank spatially-varying conv).

Computes, for full inputs
    x            [8, 32, 256, 256]  f32
    conv_w       [192, 32, 3, 3]    f32   (192 = RANK(3) * C_OUT(64))
    kernel_weight[2, 256, 256]      f32
the reference:
    y   = conv2d(x, conv_w, stride 1, pad 1)      # [8, 192, 256, 256]
    y   = y.reshape(8, 3, 64, 256, 256)
    out = y[:,0] + kw[0]*y[:,1] + kw[1]*y[:,2]    # [8, 64, 256, 256]

Strategy: spatial (H) sharding across 8 cores - each core computes a band of
32 output rows for ALL batches, so the per-pixel blend weights (which are
batch-independent) are loaded once per core and reused 8x.

Design (measured ~152-163 us HW exec across runs vs 364 us for the
full-array f32r baseline; rel err 4.2e-3 vs the 2e-2 gate):
  - bf16 inputs/weights (host-converted; f32 PSUM accumulation).
  - PE column tiling (128x64 mode): every matmul has M=64 and targets one
    PSUM partition half; the two column tiles stream CONCURRENTLY, so a
    [96,64,512] pair completes in the time of one [96,128,512]. The PE does
    ONLY the 18 conv matmuls per supertile (9 concurrent pair-slots,
    ~2.2us) - the floor for this decomposition: 2 M-tiles x 3 K-steps
    cannot shrink (K=288 > 2x128), and fp8 DoubleRow breaks the accuracy
    gate (measured 3.7e-2).
  - Per supertile t (4 image rows = 2 blocks q of 512 px): accumulate
    rank r1 -> AB[:, 0:512], r2 -> AB[:, 512:1024], r0 -> C; PSUM ends up
    rank-aligned on partition (64q+c) with no cross-partition traffic, so
    no identity-matmul folds and no transpose are needed anywhere.
  - blend fold spread across the otherwise-idle engines:
      DVE:    m = AB * svAB      [128,1024] psum*sbuf->sbuf  (~1.15us)
      DVE:    s = C + m1         [128,512]  psum+sbuf->sbuf  (~0.69us)
      GPSIMD: out_sb = s + m2    [128,512]  sbuf, bf16 out   (~1.26us)
  - software-pipelined: supertile t-1's fold ops are emitted after
    supertile t's conv matmuls, so the PE never waits on the DVE multiply;
    deep tmp/out buffering rides out transient GPSIMD/DMA lag.
  - DMA: input band for batch b+1 prefetched before batch b's output DMAs
    enter the queues; batch 0's band is loaded head-first (rows 0-7, then
    8-31) so the first matmul starts at ~+12us instead of ~+34us.
    dma_start count is kept low on purpose - each costs ~0.6us of serial
    descriptor-gen on the Sync engine (finer splitting measurably
    regresses: Sync saturates and the PE drops out of its fast p-state).
"""

import os

import numpy as np
from ml_dtypes import bfloat16 as np_bf16

B, C_IN, C_OUT, RANK, IMG = 8, 32, 64, 3, 256
N_CORES = 8
BAND = IMG // N_CORES          # 32 output rows per core
WP = IMG + 2                   # padded width 258
ROWS_IN = BAND + 2             # input rows needed per band (with halo)
SUPER = 8                      # supertiles per (batch, band): 4 rows each
SROWS = BAND // SUPER          # 4 image rows per supertile
NBLK = 512                     # pixels per matmul block (2 image rows)

_F32 = np.float32

NB = int(os.environ.get("KERNEL_NB", str(B)))  # batches to process (debug knob)


def _build_bass():
    import concourse.mybir as mybir
    import concourse.tile as tile
    from concourse import bacc

    f32 = mybir.dt.float32
    bf16 = mybir.dt.bfloat16
    nc = bacc.Bacc("TRN2", target_bir_lowering=False, debug=False)

    xs_t = nc.dram_tensor("xs", (B, C_IN, ROWS_IN * WP), bf16, kind="ExternalInput")
    # wc[(kh,cin), (r,kw), c]: 9 column-tile stationaries of 64 channels
    wc_t = nc.dram_tensor("wc", (96, 9, 64), bf16, kind="ExternalInput")
    # svAB[(q,c), t, (s,j)]: per-pixel blend weights for ranks 1 (s=0), 2 (s=1)
    svb_t = nc.dram_tensor("svb", (128, SUPER, 2 * NBLK), bf16, kind="ExternalInput")
    out_t = nc.dram_tensor("out", (B, C_OUT, BAND, IMG), bf16, kind="ExternalOutput")

    xs = xs_t.ap()
    out_r = out_t.ap().rearrange(
        "b c (t q r) w -> b q c t (r w)", t=SUPER, q=2, r=SROWS // 2
    )

    with tile.TileContext(nc) as tc:
        with (
            tc.tile_pool(name="const", bufs=1) as cpool,
            tc.tile_pool(name="imcol", bufs=3) as ipool,
            tc.tile_pool(name="psab", bufs=3, space="PSUM") as abpool,
            tc.tile_pool(name="psc", bufs=2, space="PSUM") as cpool_ps,
            tc.tile_pool(name="tmp", bufs=8) as tpool,
            tc.tile_pool(name="outp", bufs=8) as opool,
        ):
            wc_sb = cpool.tile([96, 9, 64], bf16)
            nc.sync.dma_start(wc_sb[:], wc_t.ap())

            def load_imcol(b, nchunk=1):
                # nchunk>1 only for the startup load (more queue parallelism);
                # steady-state prefetches stay coarse to spare the Sync engine
                ch = BAND * WP // nchunk
                t = ipool.tile([96, BAND * WP], bf16, tag="imcol")
                for kh in range(3):
                    for j in range(nchunk):
                        nc.sync.dma_start(
                            t[32 * kh : 32 * kh + 32, ch * j : ch * (j + 1)],
                            xs[b, :, kh * WP + ch * j : kh * WP + ch * (j + 1)],
                        )
                return t

            # batch 0's band is loaded as a small head tile (output rows 0-7,
            # enough for supertiles 0-1) plus a body tile (rows 8-31), so the
            # first matmuls start after ~400KB of DMA instead of 1.6MB; the
            # kh-baked flat layout makes the split exact (no halo overlap)
            HROWS = 2 * SROWS
            head_sb = cpool.tile([96, HROWS * WP], bf16, tag="imh")
            for kh in range(3):
                nc.sync.dma_start(
                    head_sb[32 * kh : 32 * kh + 32, :],
                    xs[0, :, kh * WP : kh * WP + HROWS * WP],
                )

            # sv tiles: 0/1 split-loaded right behind the head tile (they
            # gate the first blend multiplies), the rest after the body tile
            svb_sbs = {}

            def load_svb(t, nsp=1):
                sv = cpool.tile([128, 2 * NBLK], bf16, tag=f"svb{t}")
                for j in range(nsp):
                    w = 2 * NBLK // nsp
                    nc.sync.dma_start(
                        sv[:, w * j : w * (j + 1)],
                        svb_t.ap()[:, t, w * j : w * (j + 1)],
                    )
                svb_sbs[t] = sv

            BROWS = BAND - HROWS
            body_sb = cpool.tile([96, BROWS * WP], bf16, tag="imb")
            for kh in range(3):
                nc.sync.dma_start(
                    body_sb[32 * kh : 32 * kh + 32, :],
                    xs[0, :, kh * WP + HROWS * WP : kh * WP + HROWS * WP
                       + BROWS * WP],
                )

            load_svb(0)
            load_svb(1)
            for t in range(2, SUPER):
                load_svb(t)

            def emit_conv(imv, hl):
                """18 column-tiled conv matmuls for one supertile; returns (AB, C)."""
                ab = abpool.tile([128, 2 * NBLK], f32, tag="ab")
                c = cpool_ps.tile([128, NBLK], f32, tag="c")
                # AB matmuls first, C matmuls last: the C bank (bufs=2) has
                # a WAR on the DVE s-add two supertiles back, so its first
                # write is deferred ~1.3us into the supertile
                for kw in range(3):
                    st, sp = kw == 0, kw == 2
                    for q in range(2):
                        rhs = imv[:, hl + 2 * q : hl + 2 * q + 2, kw : kw + IMG]
                        o = 64 * q
                        nc.tensor.matmul(
                            ab[o : o + 64, 0:NBLK],
                            wc_sb[:, 3 * 1 + kw, :], rhs, start=st, stop=sp,
                        )
                        nc.tensor.matmul(
                            ab[o : o + 64, NBLK : 2 * NBLK],
                            wc_sb[:, 3 * 2 + kw, :], rhs, start=st, stop=sp,
                        )
                for kw in range(3):
                    st, sp = kw == 0, kw == 2
                    for q in range(2):
                        rhs = imv[:, hl + 2 * q : hl + 2 * q + 2, kw : kw + IMG]
                        o = 64 * q
                        nc.tensor.matmul(
                            c[o : o + 64, :],
                            wc_sb[:, 3 * 0 + kw, :], rhs, start=st, stop=sp,
                        )
                return ab, c

            def emit_blend_mult(ab, t):
                """DVE: m = AB * svAB  (psum f32 x sbuf f32 -> sbuf f32)."""
                m = tpool.tile([128, 2 * NBLK], f32, tag="m")
                nc.vector.tensor_tensor(
                    m[:], ab, svb_sbs[t][:], mybir.AluOpType.mult
                )
                return m

            def emit_fold_out(c, m, b, t):
                """DVE: s = C + m1; GPSIMD: out = s + m2; DMA out."""
                s = tpool.tile([128, NBLK], f32, tag="s")
                nc.vector.tensor_tensor(s[:], c, m[:, 0:NBLK], mybir.AluOpType.add)
                out_sb = opool.tile([128, NBLK], bf16, tag="out_sb")
                nc.gpsimd.tensor_tensor(
                    out_sb[:], s[:], m[:, NBLK : 2 * NBLK], mybir.AluOpType.add
                )
                for q in range(2):
                    nc.sync.dma_start(
                        out_r[b, q, :, t, :], out_sb[64 * q : 64 * q + 64, :]
                    )

            head_v = head_sb.rearrange("p (h w) -> p h w", w=WP)
            body_v = body_sb.rearrange("p (h w) -> p h w", w=WP)

            pend = None  # (C, m, b, t) of the previous supertile
            imcol = None
            for b in range(NB):
                imcol_nxt = load_imcol(b + 1) if b + 1 < NB else None
                imv = (
                    imcol.rearrange("p (h w) -> p h w", w=WP) if b > 0 else None
                )
                for t in range(SUPER):
                    if b == 0:
                        iv, hl = (
                            (head_v, SROWS * t)
                            if t < 2
                            else (body_v, SROWS * t - HROWS)
                        )
                    else:
                        iv, hl = imv, SROWS * t
                    ab, c = emit_conv(iv, hl)
                    m = emit_blend_mult(ab, t)
                    if pend is not None:
                        emit_fold_out(*pend)
                    pend = (c, m, b, t)
                imcol = imcol_nxt
            emit_fold_out(*pend)
    nc.compile()
    return nc


_CACHE = {}


def _get_bass():
    if "nc" not in _CACHE:
        _CACHE["nc"] = _build_bass()
    return _CACHE["nc"]


def _prep_shards(x, conv_w, kernel_weight):
    x = np.asarray(x, dtype=_F32)
    conv_w = np.asarray(conv_w, dtype=_F32)
    kernel_weight = np.asarray(kernel_weight, dtype=_F32)

    x_pad = np.pad(x, ((0, 0), (0, 0), (1, 1), (1, 1))).astype(np_bf16)
    # wc[(kh,cin), (r,kw), c] from conv_w[(r c), cin, kh, kw]
    wc = np.ascontiguousarray(
        conv_w.reshape(RANK, C_OUT, C_IN, 3, 3)
        .transpose(3, 2, 0, 4, 1)
        .reshape(96, 9, 64)
    ).astype(np_bf16)

    in_maps = []
    for i in range(N_CORES):
        h0 = BAND * i
        shard = np.ascontiguousarray(
            x_pad[:, :, h0 : h0 + ROWS_IN, :]
        ).reshape(B, C_IN, ROWS_IN * WP)
        band = kernel_weight[:, h0 : h0 + BAND, :]          # [2, 32, 256]
        # svAB[64q+c, t, (s,j)] = band[s, 4t+2q+(j//256), j%256]
        tmp = band.reshape(2, SUPER, 2, NBLK)               # [s, t, q, j]
        svb = np.broadcast_to(
            tmp.transpose(2, 1, 0, 3)[:, None],             # [q, 1, t, s, j]
            (2, C_OUT, SUPER, 2, NBLK),
        ).reshape(128, SUPER, 2 * NBLK)
        svb = np.ascontiguousarray(svb).astype(np_bf16)
        in_maps.append({"xs": shard, "wc": wc, "svb": svb})
    return in_maps


def run(inputs, trace=False):
    """Run the sharded bass kernel; returns (out_full, BassKernelResults)."""
    from concourse.bass_utils import run_bass_kernel_spmd

    in_maps = _prep_shards(**inputs)
    nc = _get_bass()
    res = run_bass_kernel_spmd(
        nc, in_maps, core_ids=list(range(N_CORES)), trace=trace
    )
    out = np.empty((B, C_OUT, IMG, IMG), dtype=_F32)
    for i in range(N_CORES):
        out[:, :, BAND * i : BAND * (i + 1), :] = res.results[i]["out"]
    return out, res


def kernel(x, conv_w, kernel_weight):
    out, _ = run({"x": x, "conv_w": conv_w, "kernel_weight": kernel_weight})
    return out



# revision 2
# speedup vs baseline: 1.0191x; 1.0191x over previous
"""Trainium2 Bass kernel for nn_LRSVConv (low-rank spatially-varying conv).

Computes, for full inputs
    x            [8, 32, 256, 256]  f32
    conv_w       [192, 32, 3, 3]    f32   (192 = RANK(3) * C_OUT(64))
    kernel_weight[2, 256, 256]      f32
the reference:
    y   = conv2d(x, conv_w, stride 1, pad 1)      # [8, 192, 256, 256]
    y   = y.reshape(8, 3, 64, 256, 256)
    out = y[:,0] + kw[0]*y[:,1] + kw[1]*y[:,2]    # [8, 64, 256, 256]

Strategy: spatial (H) sharding across 8 cores - each core computes a band of
32 output rows for ALL batches, so the per-pixel blend weights (which are
batch-independent) are loaded once per core and reused 8x.

v2 design (v1 measured 150.2us; trace showed the MM body already dense at
the 216ns/pair-slot warm roofline for the 9-slot/supertile decomposition =
124.7us floor, so v2 attacks the head (12.5us to first MM, HAM cold until
18.8us) and tail (8us of serial fold+DMA after the last MM)):
  - bf16 inputs/weights (host-converted; f32 PSUM accumulation).
  - PE column tiling: per supertile (4 rows = 2 blocks q of 512 px),
    9 pair-slots of [96,64,512] matmuls: ranks 1,2 -> AB1/AB2 psum banks,
    rank 0 -> C; (q0,q1) stream concurrently per (rank,kw) pair-slot.
  - kh-baked input layout is premade IN DRAM by the host (xs[b, (kh c),
    band rows]) so each batch band is ONE dma_start (v1: 3) - the serial
    ~0.6us/dma_start descriptor-gen on Sync was gating the head.
  - batch 0's band is loaded as 8 per-supertile chunks so supertile 0 only
    waits for 198KB; warmup: 8 dummy matmuls (emitted first, no DMA deps)
    keep the PE busy from main-start so the HAM clock-gate (K=4/8 cold ->
    8/8 warm after ~3.4us busy) is released by the time real MMs start.
  - fold spread over idle engines, split per rank so it starts early:
      DVE:    m1 = AB1 * sv1, m2 = AB2 * sv2, s = C + m1   (3x [128,512])
      GPSIMD: out_sb = s + m2 (bf16 out; DVE for the last supertile - the
              tail chain after the final C matmul is s -> add -> dma)
  - output written as [b, t, (q c), (r w)] so each supertile is ONE
    contiguous [128,512] dma_start (v1: 2); host un-shuffles at gather.
"""

import os

import numpy as np
from ml_dtypes import bfloat16 as np_bf16

B, C_IN, C_OUT, RANK, IMG = 8, 32, 64, 3, 256
N_CORES = 8
BAND = IMG // N_CORES          # 32 output rows per core
WP = IMG + 2                   # padded width 258
SUPER = 8                      # supertiles per (batch, band): 4 rows each
SROWS = BAND // SUPER          # 4 image rows per supertile
NBLK = 512                     # pixels per matmul block (2 image rows)
N_WARM = 8                     # dummy PE warmup matmuls (HAM un-throttle)

_F32 = np.float32

NB = int(os.environ.get("KERNEL_NB", str(B)))  # batches to process (debug knob)


def _build_bass():
    import concourse.mybir as mybir
    import concourse.tile as tile
    from concourse import bacc

    f32 = mybir.dt.float32
    bf16 = mybir.dt.bfloat16
    nc = bacc.Bacc("TRN2", target_bir_lowering=False, debug=False)

    # xs[b, (kh,cin), (r,w)]: kh-shifted copies premade on host; row r of
    # copy kh is padded-input row (band_start + r + kh), all 258 cols
    xs_t = nc.dram_tensor("xs", (B, 96, BAND * WP), bf16, kind="ExternalInput")
    # wc[(kh,cin), (r,kw), c]: 9 column-tile stationaries of 64 channels
    wc_t = nc.dram_tensor("wc", (96, 9, 64), bf16, kind="ExternalInput")
    # svb[(q,c), t, (s,j)]: per-pixel blend weights for ranks 1 (s=0), 2 (s=1)
    svb_t = nc.dram_tensor("svb", (128, SUPER, 2 * NBLK), bf16, kind="ExternalInput")
    # out[b, t, (q,c), (r,w)]: supertile-contiguous; host unshuffles
    out_t = nc.dram_tensor("out", (B, SUPER, 128, NBLK), bf16, kind="ExternalOutput")

    xs = xs_t.ap()
    out_ap = out_t.ap()

    with tile.TileContext(nc) as tc:
        with (
            tc.tile_pool(name="const", bufs=1) as cpool,
            tc.tile_pool(name="imcol", bufs=3) as ipool,
            tc.tile_pool(name="ps", bufs=3, space="PSUM") as pspool,
            tc.tile_pool(name="tmp", bufs=6) as tpool,
            tc.tile_pool(name="outp", bufs=8) as opool,
        ):
            # --- PE warmup: emitted first so the Tensor queue starts on it
            # at main-start (no DMA deps); releases the HAM clock gate
            # (~3.4us sustained busy) before the first real matmul arrives.
            dummy = cpool.tile([128, NBLK], bf16, tag="dummy")
            nc.vector.memset(dummy[:], 0.0)
            wps = pspool.tile([128, NBLK], f32, tag="ab1", name="wps")
            for _ in range(N_WARM):
                nc.tensor.matmul(
                    wps[0:64, :], dummy[0:96, 0:64], dummy[0:96, :],
                    start=True, stop=True,
                )

            # --- DMA gen order = Sync program order; head-critical first.
            wc_sb = cpool.tile([96, 9, 64], bf16)
            nc.sync.dma_start(wc_sb[:], wc_t.ap())

            # batch 0's band in per-supertile chunks (supertile 0 gated by
            # 198KB instead of 1.6MB)
            b0c = []
            for t in range(SUPER):
                ch = cpool.tile([96, SROWS * WP], bf16, tag=f"b0c{t}", name="ch")
                b0c.append(ch)

            def load_b0(t):
                nc.sync.dma_start(
                    b0c[t][:], xs[0, :, SROWS * t * WP : SROWS * (t + 1) * WP]
                )

            svb_sb = cpool.tile([128, SUPER, 2 * NBLK], bf16)

            load_b0(0)
            nc.sync.dma_start(svb_sb[:, 0:2, :], svb_t.ap()[:, 0:2, :])
            load_b0(1)
            load_b0(2)
            nc.sync.dma_start(svb_sb[:, 2:SUPER, :], svb_t.ap()[:, 2:SUPER, :])
            for t in range(3, SUPER):
                load_b0(t)

            def load_imcol(b):
                t = ipool.tile([96, BAND * WP], bf16, tag="imcol")
                nc.sync.dma_start(t[:], xs[b, :, :])
                return t

            def emit_conv(iv, hl):
                """18 column-tiled conv matmuls for one supertile."""
                ab1 = pspool.tile([128, NBLK], f32, tag="ab1", name="ab1")
                ab2 = pspool.tile([128, NBLK], f32, tag="ab2", name="ab2")
                c = pspool.tile([128, NBLK], f32, tag="c", bufs=2, name="c")
                # rank 1 first: its 6 matmuls complete 2/3 of a supertile
                # early, so the DVE m1 multiply overlaps the r2/C matmuls.
                # C last: its bank (bufs=2) has a WAR on the s-add one
                # supertile back. (q0,q1) adjacent -> column-tile pairs.
                for r, ps in ((1, ab1), (2, ab2), (0, c)):
                    for kw in range(3):
                        st, sp = kw == 0, kw == 2
                        for q in range(2):
                            rhs = iv[:, hl + 2 * q : hl + 2 * q + 2, kw : kw + IMG]
                            o = 64 * q
                            nc.tensor.matmul(
                                ps[o : o + 64, :],
                                wc_sb[:, 3 * r + kw, :], rhs, start=st, stop=sp,
                            )
                return ab1, ab2, c

            def emit_fold(ab1, ab2, c, b, t, last=False):
                """DVE: m1,m2,s; GPSIMD (DVE if last): out add; 1 dma out."""
                m1 = tpool.tile([128, NBLK], f32, tag="m1", name="m1")
                nc.vector.tensor_tensor(
                    m1[:], ab1[:], svb_sb[:, t, 0:NBLK], mybir.AluOpType.mult
                )
                m2 = tpool.tile([128, NBLK], f32, tag="m2", name="m2")
                nc.vector.tensor_tensor(
                    m2[:], ab2[:], svb_sb[:, t, NBLK : 2 * NBLK],
                    mybir.AluOpType.mult,
                )
                s = tpool.tile([128, NBLK], f32, tag="s", name="s")
                nc.vector.tensor_tensor(s[:], c[:], m1[:], mybir.AluOpType.add)
                out_sb = opool.tile([128, NBLK], bf16, tag="out_sb", name="out_sb")
                eng = nc.vector if last else nc.gpsimd
                eng.tensor_tensor(out_sb[:], s[:], m2[:], mybir.AluOpType.add)
                nc.sync.dma_start(out_ap[b, t], out_sb[:])

            imcol = None
            for b in range(NB):
                imcol_nxt = load_imcol(b + 1) if b + 1 < NB else None
                imv = (
                    imcol.rearrange("p (h w) -> p h w", w=WP) if b > 0 else None
                )
                for t in range(SUPER):
                    if b == 0:
                        iv, hl = b0c[t].rearrange("p (h w) -> p h w", w=WP), 0
                    else:
                        iv, hl = imv, SROWS * t
                    ab1, ab2, c = emit_conv(iv, hl)
                    emit_fold(
                        ab1, ab2, c, b, t,
                        last=(b == NB - 1 and t == SUPER - 1),
                    )
                imcol = imcol_nxt
    nc.compile()
    return nc


_CACHE = {}


def _get_bass():
    if "nc" not in _CACHE:
        _CACHE["nc"] = _build_bass()
    return _CACHE["nc"]


def _prep_shards(x, conv_w, kernel_weight):
    x = np.asarray(x, dtype=_F32)
    conv_w = np.asarray(conv_w, dtype=_F32)
    kernel_weight = np.asarray(kernel_weight, dtype=_F32)

    x_pad = np.pad(x, ((0, 0), (0, 0), (1, 1), (1, 1))).astype(np_bf16)
    # wc[(kh,cin), (r,kw), c] from conv_w[(r c), cin, kh, kw]
    wc = np.ascontiguousarray(
        conv_w.reshape(RANK, C_OUT, C_IN, 3, 3)
        .transpose(3, 2, 0, 4, 1)
        .reshape(96, 9, 64)
    ).astype(np_bf16)

    in_maps = []
    for i in range(N_CORES):
        h0 = BAND * i
        # xs[b, (kh c), (r w)] = x_pad[b, c, h0 + r + kh, w]
        shard = np.ascontiguousarray(
            np.stack(
                [x_pad[:, :, h0 + kh : h0 + kh + BAND, :] for kh in range(3)],
                axis=1,
            )
        ).reshape(B, 96, BAND * WP)
        band = kernel_weight[:, h0 : h0 + BAND, :]          # [2, 32, 256]
        # svb[64q+c, t, (s,j)] = band[s, 4t+2q+(j//256), j%256]
        tmp = band.reshape(2, SUPER, 2, NBLK)               # [s, t, q, j]
        svb = np.broadcast_to(
            tmp.transpose(2, 1, 0, 3)[:, None],             # [q, 1, t, s, j]
            (2, C_OUT, SUPER, 2, NBLK),
        ).reshape(128, SUPER, 2 * NBLK)
        svb = np.ascontiguousarray(svb).astype(np_bf16)
        in_maps.append({"xs": shard, "wc": wc, "svb": svb})
    return in_maps


def run(inputs, trace=False):
    """Run the sharded bass kernel; returns (out_full, BassKernelResults)."""
    from concourse.bass_utils import run_bass_kernel_spmd

    in_maps = _prep_shards(**inputs)
    nc = _get_bass()
    res = run_bass_kernel_spmd(
        nc, in_maps, core_ids=list(range(N_CORES)), trace=trace
    )
    out = np.empty((B, C_OUT, IMG, IMG), dtype=_F32)
    for i in range(N_CORES):
        # res: [B, SUPER, (q c), (r w)] -> [B, c, (t q r), w]
        band = (
            np.asarray(res.results[i]["out"], dtype=_F32)
            .reshape(B, SUPER, 2, C_OUT, 2, IMG)
            .transpose(0, 3, 1, 2, 4, 5)
            .reshape(B, C_OUT, BAND, IMG)
        )
        out[:, :, BAND * i : BAND * (i + 1), :] = band
    return out, res


def kernel(x, conv_w, kernel_weight):
    out, _ = run({"x": x, "conv_w": conv_w, "kernel_weight": kernel_weight})
    return out
